# revision 1
# baseline (speedup 1.0000x reference)
"""Trainium2 Bass kernel for BaselineMoE (top-6-of-32 routed experts + 2 shared).

Strategy (8 NeuronCores, expert-parallel per the sharding hint):
  - Host computes the (cheap) router softmax/top-k from the actual inputs,
    gathers each expert's tokens into a padded, transposed buffer, and deals
    the 32 routed experts across 8 cores x 4 slots, balancing per-core load.
  - Each core runs a dense SwiGLU MLP (gate/up/down, sigmoid(gate)*up) for its
    4 routed experts on the pre-gathered tokens, with the per-token top-k gate
    weights applied on-device during PSUM evacuation.
  - Routed experts run in fp8e4 with DoubleRow matmuls (2 contraction rows per
    PE cell) using power-of-2 scales folded into the sigmoid input scale and
    the gate coefficients; PSUM accumulation stays f32.
  - The 2 shared experts are split across core halves (cores 0-3 run shared
    expert 0, cores 4-7 expert 1, each on a 512-token shard) in bf16 — they
    carry most of the output magnitude, so they stay higher precision, and the
    split halves the shared-weight HBM traffic vs full replication.
  - Each tensor is loaded/stored with a single large rearranged-AP DMA
    (~32 DMAs per pass total) to keep the DGE issue path off the critical
    path. Expert outputs come back bf16; the host scatter-adds them into the
    residual stream in f32.

Capacities (per-slot token counts) are computed from the actual routing at
call time, so the emitted program adapts to the input.
"""

from contextlib import ExitStack

import numpy as np
import ml_dtypes

import concourse.bacc as bacc
import concourse.tile as tile
import concourse.mybir as mybir
from concourse.bass_utils import run_bass_kernel_spmd

H = 2048
I = 1024
E = 32
NS = 2
TOP_K = 6
SCALE = 1.0
NCORES = 8
SLOTS = 4          # routed experts per core
TSH = 512          # shared-expert tokens per core (T / 4; 2-way expert split)
KH = H // 128      # 16 k-tiles over H
KI = I // 128      # 8 k-tiles over I
PH = H // 256      # 8 double-row pairs over H
PI = I // 256      # 4 double-row pairs over I
BF16 = mybir.dt.bfloat16
F32 = mybir.dt.float32
FP8 = mybir.dt.float8e4
NP_FP8 = mybir.dt.np(FP8)

# power-of-2 fp8 scales (descales are folded into sigmoid scale / gates).
# fp8e4 here is IEEE e4m3 (max finite 240): z = sigmoid(g) * u carries
# S_WU * S_X = 32x and must stay well under 240 when cast to fp8.
S_X = 8.0          # tokens
S_WG = 8.0         # gate weights
S_WU = 4.0         # up weights
S_WD = 32.0        # down weights
DESCALE_GATE = 1.0 / (S_WG * S_X)                    # on sigmoid input
S_Y = 64.0         # fp8 y-output scale (divided out on host)
DESCALE_Y = S_Y / (S_WU * S_X * S_WD)                # folded into gates

_PROGRAM_CACHE: dict = {}


def _to_bf16(a: np.ndarray) -> np.ndarray:
    """f32 -> bf16 with round-to-nearest-even (fast uint trick)."""
    a = np.ascontiguousarray(a, dtype=np.float32)
    u = a.view(np.uint32)
    r = (u + np.uint32(0x7FFF) + ((u >> np.uint32(16)) & np.uint32(1))) >> np.uint32(16)
    return r.astype(np.uint16).view(ml_dtypes.bfloat16)


def _fp8_pairs(a: np.ndarray, scale: float) -> np.ndarray:
    """[K, N] f32 -> [K/256, 128, 2, N] fp8e4, DoubleRow-interleaved."""
    K, N = a.shape
    q = (np.asarray(a, np.float32) * scale).reshape(K // 256, 2, 128, N)
    return np.ascontiguousarray(q.transpose(0, 2, 1, 3)).astype(NP_FP8)


def _route(flat: np.ndarray, Wr: np.ndarray):
    """Host router: softmax over experts, exact top-k gate mask."""
    logits = flat.astype(np.float32) @ Wr.astype(np.float32)
    m = logits.max(axis=-1, keepdims=True)
    p = np.exp(logits - m)
    p /= p.sum(axis=-1, keepdims=True)
    T = p.shape[0]
    idx = np.argpartition(-p, TOP_K - 1, axis=-1)[:, :TOP_K]
    gates = np.zeros((T, E), np.float32)
    rows = np.arange(T)[:, None]
    gates[rows, idx] = p[rows, idx] * SCALE
    return gates


def _assign_experts(tok_idx):
    """Deal experts into (core, slot) balancing per-core token totals.

    Experts with more than 512 tokens (the PSUM-bank N limit) are split into
    pseudo-experts with disjoint token chunks, so slot capacity never exceeds
    512. Slot s holds the pseudo-experts ranked [8s, 8s+8) by token count;
    within a slot the largest goes to the least-loaded core. Returns
    (assign, caps, chunks) where chunks[j] = (expert, token_index_array) and
    assign[core][slot] indexes into chunks (-1 = empty).
    """
    chunks = []
    for e, ix in enumerate(tok_idx):
        for off in range(0, max(len(ix), 1), 512):
            chunks.append((e, ix[off:off + 512]))
    while len(chunks) % NCORES:
        chunks.append((0, np.zeros(0, np.int32)))
    counts = np.array([len(ix) for _, ix in chunks], np.int64)
    n_slots = len(chunks) // NCORES
    order = np.argsort(-counts, kind="stable")
    assign = [[-1] * n_slots for _ in range(NCORES)]
    load = np.zeros(NCORES, np.int64)
    caps = []
    for s in range(n_slots):
        group = list(order[s * NCORES:(s + 1) * NCORES])
        caps.append(int(counts[group].max()) if group else 0)
        for j in group:  # descending count; give to least-loaded core
            c = int(np.argmin(load))
            assign[c][s] = int(j)
            load[c] += counts[j]
    caps = [min(512, max(64, -(-c // 16) * 16)) for c in caps]
    return assign, caps, chunks


def build_program(caps, loop_reps=None, parts="all"):
    """Build the per-core Bass program for the given slot capacities.

    loop_reps: if set, wrap the whole body in a device-side For_i loop —
    used by the test harness to amplify exec time above dispatch overhead.
    """
    caps = tuple(int(c) for c in caps)
    key = (caps, loop_reps, parts)
    if key in _PROGRAM_CACHE:
        return _PROGRAM_CACHE[key]

    nc = bacc.Bacc("TRN2", target_bir_lowering=False, debug=False)

    xg_d, wg_d, wu_d, wd_d, g_d, y_d = [], [], [], [], [], []
    for s in range(len(caps)):
        C = caps[s]
        xg_d.append(nc.dram_tensor(f"xg{s}", [PH, 128, 2, C], FP8, kind="ExternalInput"))
        wg_d.append(nc.dram_tensor(f"wg{s}", [PH, 128, 2, I], FP8, kind="ExternalInput"))
        wu_d.append(nc.dram_tensor(f"wu{s}", [PH, 128, 2, I], FP8, kind="ExternalInput"))
        wd_d.append(nc.dram_tensor(f"wd{s}", [PI, 128, 2, H], FP8, kind="ExternalInput"))
        g_d.append(nc.dram_tensor(f"g{s}", [1, C], BF16, kind="ExternalInput"))
        y_d.append(nc.dram_tensor(f"y{s}", [KH, 128, C], FP8, kind="ExternalOutput"))
    xs_d = nc.dram_tensor("xs", [KH, 128, TSH], BF16, kind="ExternalInput")
    wgs_d = nc.dram_tensor("wgs", [KH, 128, I], BF16, kind="ExternalInput")
    wus_d = nc.dram_tensor("wus", [KH, 128, I], BF16, kind="ExternalInput")
    wds_d = nc.dram_tensor("wds", [KI, 128, H], BF16, kind="ExternalInput")
    ys_d = nc.dram_tensor("ys", [KH, 128, TSH], BF16, kind="ExternalOutput")

    DR = mybir.MatmulPerfMode.DoubleRow

    with tile.TileContext(nc) as tc:
        with (
            tc.tile_pool(name="w", bufs=5) as wpool,
            tc.tile_pool(name="xg", bufs=4) as xpool,
            tc.tile_pool(name="gb", bufs=2) as gbpool,
            tc.tile_pool(name="sg", bufs=2) as sgpool,
            tc.tile_pool(name="z", bufs=2) as zpool,
            tc.tile_pool(name="o", bufs=2) as opool,
            tc.tile_pool(name="os", bufs=2) as ospool,
            tc.tile_pool(name="pg", bufs=2, space="PSUM") as pgpool,
            tc.tile_pool(name="pu", bufs=2, space="PSUM") as pupool,
            tc.tile_pool(name="py", bufs=4, space="PSUM") as pypool,
            ExitStack() as stack,
        ):
            if loop_reps is not None:
                stack.enter_context(tc.For_i(0, loop_reps, 1))

            st = {}

            def routed_expert(s):
                C = caps[s]
                xg_t = xpool.tile([128, PH, 2, C], FP8, tag="xg", name=f"xg_t{s}")
                for hf in range(2):
                    nc.sync.dma_start(
                        xg_t[:, 4 * hf:4 * hf + 4],
                        xg_d[s][4 * hf:4 * hf + 4].rearrange("p q r c -> q p r c"))
                gb = gbpool.tile([128, C], BF16, tag="gb", name=f"gb{s}")
                nc.sync.dma_start(gb[:], g_d[s][:].partition_broadcast(128))

                wg_t = wpool.tile([128, PH, 2, I], FP8, tag="w", name=f"wg_t{s}")
                for hf in range(2):
                    nc.sync.dma_start(
                        wg_t[:, 4 * hf:4 * hf + 4],
                        wg_d[s][4 * hf:4 * hf + 4].rearrange("p q r i -> q p r i"))
                sg = sgpool.tile([128, KI, C], BF16, tag="sg", name=f"sg{s}")
                for m in range(KI):
                    pg = pgpool.tile([128, C], F32, tag="pg", name=f"pg{s}_{m}")
                    for p in range(PH):
                        nc.tensor.matmul(pg[:], wg_t[:, p, :, m * 128:(m + 1) * 128],
                                         xg_t[:, p], start=(p == 0), stop=(p == PH - 1),
                                         perf_mode=DR)
                    nc.scalar.activation(sg[:, m, :], pg[:],
                                         mybir.ActivationFunctionType.Sigmoid,
                                         scale=DESCALE_GATE)

                wu_t = wpool.tile([128, PH, 2, I], FP8, tag="w", name=f"wu_t{s}")
                for hf in range(2):
                    nc.sync.dma_start(
                        wu_t[:, 4 * hf:4 * hf + 4],
                        wu_d[s][4 * hf:4 * hf + 4].rearrange("p q r i -> q p r i"))
                z = zpool.tile([128, KI, C], FP8, tag="z", name=f"z{s}")
                for m in range(KI):
                    pu = pupool.tile([128, C], F32, tag="pu", name=f"pu{s}_{m}")
                    for p in range(PH):
                        nc.tensor.matmul(pu[:], wu_t[:, p, :, m * 128:(m + 1) * 128],
                                         xg_t[:, p], start=(p == 0), stop=(p == PH - 1),
                                         perf_mode=DR)
                    nc.vector.tensor_mul(z[:, m, :], sg[:, m, :], pu[:])

                wd_t = wpool.tile([128, PI, 2, H], FP8, tag="w", name=f"wd_t{s}")
                for hf in range(2):
                    nc.sync.dma_start(
                        wd_t[:, 2 * hf:2 * hf + 2],
                        wd_d[s][2 * hf:2 * hf + 2].rearrange("p q r h -> q p r h"))
                ot = opool.tile([128, KH, C], FP8, tag="o", name=f"ot{s}")
                for h in range(KH):
                    py = pypool.tile([128, C], F32, tag="py", name=f"py{s}_{h}")
                    for p in range(PI):
                        nc.tensor.matmul(py[:], wd_t[:, p, :, h * 128:(h + 1) * 128],
                                         z[:, 2 * p:2 * p + 2, :], start=(p == 0),
                                         stop=(p == PI - 1), perf_mode=DR)
                    nc.vector.tensor_mul(ot[:, h, :], py[:], gb[:])
                for hf in range(2):
                    nc.sync.dma_start(
                        y_d[s][8 * hf:8 * hf + 8].rearrange("h q c -> q h c"),
                        ot[:, 8 * hf:8 * hf + 8])

            # shared-expert sub-phases (bf16, one expert per core half);
            # tiles split into k-halves so each load is an independent DMA
            # in a uniform 16KB/partition weight-pool slot
            def shared_p1():
                xs_t = [xpool.tile([128, KH // 2, TSH], BF16, tag="xg",
                                   name=f"xs_t{hf}") for hf in range(2)]
                for hf in range(2):
                    nc.sync.dma_start(
                        xs_t[hf][:],
                        xs_d[8 * hf:8 * hf + 8].rearrange("k q t -> q k t"))
                wg_t = [wpool.tile([128, KH // 2, I], BF16, tag="w",
                                   name=f"wgs_t{hf}") for hf in range(2)]
                for hf in range(2):
                    nc.sync.dma_start(
                        wg_t[hf][:],
                        wgs_d[8 * hf:8 * hf + 8].rearrange("k q i -> q k i"))
                sgs = sgpool.tile([128, KI, TSH], BF16, tag="sg")
                for m in range(KI):
                    pg = pgpool.tile([128, TSH], F32, tag="pg", name=f"pgs_{m}")
                    for k in range(KH):
                        nc.tensor.matmul(pg[:],
                                         wg_t[k // 8][:, k % 8, m * 128:(m + 1) * 128],
                                         xs_t[k // 8][:, k % 8, :],
                                         start=(k == 0), stop=(k == KH - 1))
                    nc.scalar.activation(sgs[:, m, :], pg[:],
                                         mybir.ActivationFunctionType.Sigmoid)
                st["xs_t"] = xs_t
                st["sgs"] = sgs

            def shared_p2():
                xs_t, sgs = st["xs_t"], st["sgs"]
                wu_t = [wpool.tile([128, KH // 2, I], BF16, tag="w",
                                   name=f"wus_t{hf}") for hf in range(2)]
                for hf in range(2):
                    nc.sync.dma_start(
                        wu_t[hf][:],
                        wus_d[8 * hf:8 * hf + 8].rearrange("k q i -> q k i"))
                zb = zpool.tile([128, KI, TSH], BF16, tag="z")
                for m in range(KI):
                    pu = pupool.tile([128, TSH], F32, tag="pu", name=f"pus_{m}")
                    for k in range(KH):
                        nc.tensor.matmul(pu[:],
                                         wu_t[k // 8][:, k % 8, m * 128:(m + 1) * 128],
                                         xs_t[k // 8][:, k % 8, :],
                                         start=(k == 0), stop=(k == KH - 1))
                    nc.vector.tensor_mul(zb[:, m, :], sgs[:, m, :], pu[:])
                st["zb"] = zb

            def shared_p3(half):
                zb = st["zb"]
                if half == 0:
                    wd_t = [wpool.tile([128, KI // 2, H], BF16, tag="w",
                                       name=f"wds_t{hf}") for hf in range(2)]
                    for hf in range(2):
                        nc.sync.dma_start(
                            wd_t[hf][:],
                            wds_d[4 * hf:4 * hf + 4].rearrange("j q h -> q j h"))
                    st["wd_t"] = wd_t
                wd_t = st["wd_t"]
                for hg in range(2 * half, 2 * half + 2):
                    os_t = ospool.tile([128, 4, TSH], BF16, tag="os",
                                       name=f"os_t{hg}")
                    for hh in range(4):
                        h = hg * 4 + hh
                        py = pypool.tile([128, TSH], F32, tag="py", name=f"pys_{h}")
                        for j in range(KI):
                            nc.tensor.matmul(py[:],
                                             wd_t[j // 4][:, j % 4, h * 128:(h + 1) * 128],
                                             zb[:, j, :], start=(j == 0),
                                             stop=(j == KI - 1))
                        nc.vector.tensor_copy(os_t[:, hh, :], py[:])
                    nc.sync.dma_start(
                        ys_d[hg * 4:(hg + 1) * 4].rearrange("h q t -> q h t"),
                        os_t[:])

            # interleave shared sub-phases between routed experts: shared is
            # PE-bound while routed is DMA-bound, so alternating them lets
            # each phase's idle resource cover the other's busy one
            if parts == "routed":
                steps = [lambda s=s: routed_expert(s) for s in range(len(caps))]
            elif parts == "shared":
                steps = [shared_p1, shared_p2,
                         lambda: shared_p3(0), lambda: shared_p3(1)]
            elif parts == "seq":
                steps = [lambda s=s: routed_expert(s) for s in range(len(caps))]
                steps += [shared_p1, shared_p2,
                          lambda: shared_p3(0), lambda: shared_p3(1)]
            else:
                shared_steps = [shared_p1, shared_p2,
                                lambda: shared_p3(0), lambda: shared_p3(1)]
                steps = []
                for s in range(len(caps)):
                    steps.append(lambda s=s: routed_expert(s))
                    if s < len(shared_steps):
                        steps.append(shared_steps[s])
                steps.extend(shared_steps[len(caps):])
            for step in steps:
                step()

    nc.compile()
    _PROGRAM_CACHE[key] = nc
    return nc


def prepare(x, Wr, Wg_s, Wu_s, Wd_s, Wg, Wu, Wd):
    """Host-side routing, sharding and fp8/bf16 packing. Returns (nc, in_maps, meta)."""
    flat = np.ascontiguousarray(x, np.float32).reshape(-1, H)
    T = flat.shape[0]
    assert T == 4 * TSH

    gates = _route(flat, Wr)
    tok_idx = [np.nonzero(gates[:, e])[0].astype(np.int32) for e in range(E)]
    assign, caps, chunks = _assign_experts(tok_idx)

    nc = build_program(caps)

    xT = np.ascontiguousarray(flat.T)          # [H, T] f32
    wgs_b = [_to_bf16(Wg_s[e]).reshape(KH, 128, I) for e in range(NS)]
    wus_b = [_to_bf16(Wu_s[e]).reshape(KH, 128, I) for e in range(NS)]
    wds_b = [_to_bf16(Wd_s[e]).reshape(KI, 128, H) for e in range(NS)]
    xs_b = [_to_bf16(xT[:, p * TSH:(p + 1) * TSH]).reshape(KH, 128, TSH)
            for p in range(4)]

    in_maps = []
    for c in range(NCORES):
        half, part = divmod(c, 4)
        im = {"wgs": wgs_b[half], "wus": wus_b[half], "wds": wds_b[half],
              "xs": xs_b[part]}
        for s in range(len(caps)):
            e, ix = chunks[assign[c][s]]
            C = caps[s]
            xg = np.zeros((H, C), np.float32)
            xg[:, :len(ix)] = xT[:, ix]
            im[f"xg{s}"] = _fp8_pairs(xg, S_X)
            g = np.zeros((1, C), np.float32)
            g[0, :len(ix)] = gates[ix, e] * DESCALE_Y
            im[f"g{s}"] = _to_bf16(g)
            im[f"wg{s}"] = _fp8_pairs(np.asarray(Wg[e]), S_WG)
            im[f"wu{s}"] = _fp8_pairs(np.asarray(Wu[e]), S_WU)
            im[f"wd{s}"] = _fp8_pairs(np.asarray(Wd[e]), S_WD)
        in_maps.append(im)

    meta = {"assign": assign, "caps": caps, "chunks": chunks,
            "flat": flat, "shape": x.shape}
    return nc, in_maps, meta


def postprocess(results, meta):
    """Scatter-add per-expert outputs + shared shards + residual."""
    flat = meta["flat"]
    out = flat.copy()
    for c in range(NCORES):
        part = c % 4
        sh = results[c]["ys"].reshape(H, TSH).astype(np.float32)
        out[part * TSH:(part + 1) * TSH] += sh.T
        for s in range(len(meta["caps"])):
            _, ix = meta["chunks"][meta["assign"][c][s]]
            if len(ix) == 0:
                continue
            Y = results[c][f"y{s}"].reshape(H, meta["caps"][s])
            out[ix] += Y[:, :len(ix)].T.astype(np.float32) * (1.0 / S_Y)
    return out.reshape(meta["shape"]).astype(np.float32, copy=False)


def kernel(x, Wr, Wg_s, Wu_s, Wd_s, Wg, Wu, Wd):
    nc, in_maps, meta = prepare(x, Wr, Wg_s, Wu_s, Wd_s, Wg, Wu, Wd)
    last_err = None
    for _ in range(3):  # the tunneled device occasionally drops a run
        try:
            res = run_bass_kernel_spmd(nc, in_maps, list(range(NCORES)))
            return postprocess(res.results, meta)
        except Exception as err:  # noqa: BLE001
            last_err = err
    raise last_err



# revision 15
# speedup vs baseline: 1.1590x; 1.1590x over previous
"""Trainium2 Bass kernel for BaselineMoE (top-6-of-32 routed experts + 2 shared).

Strategy (8 NeuronCores, expert-parallel per the sharding hint):
  - Host computes the (cheap) router softmax/top-k from the actual inputs,
    gathers each expert's tokens into a padded, transposed buffer, and deals
    the 32 routed experts across 8 cores x 4 slots, balancing per-core load
    (sum of slot capacities is provably minimal for 4 slots x 8 cores).
  - Each core runs a dense SwiGLU MLP (gate/up/down, sigmoid(gate)*up) for its
    4 routed experts on the pre-gathered tokens, with the per-token top-k gate
    weights applied on-device during PSUM evacuation.
  - Routed experts run in fp8e4 with DoubleRowSwInterleave matmuls (2
    contraction rows per PE cell, software-interleaved weight layout) using
    power-of-2 scales folded into the sigmoid input scale and the gate
    coefficients; PSUM accumulation stays f32.
  - The 2 shared experts are split across core halves (cores 0-3 run shared
    expert 0, cores 4-7 expert 1, each on a 512-token shard). Their gate/up
    matmuls run in fp8 DoubleRow (the sigmoid and the z = sig(g)*u product
    damp the quantization error); the down projection stays bf16, which keeps
    total rel-err ~1.6e-2 vs the 2e-2 budget (measured on HW). The 1/32
    activation descale is folded into the bf16 down weights host-side.
  - Phases run sequentially (all routed slots, then shared): measured ~7us
    faster than interleaving because shared tiles otherwise occupy weight-pool
    buffers across routed phases and starve the DMA prefetch depth.
  - Each tensor is loaded/stored with a few large rearranged-AP DMAs to keep
    the DGE issue path off the critical path. Expert outputs come back fp8
    (scaled by S_Y); the host scatter-adds them into the residual in f32.

Perf model (measured on these cores): PE streams ~0.54ns per output column
for both bf16 and fp8-DR, so time ~= total matmul output columns x 0.54ns;
fp8-DR halves the column count per flop (256-deep contraction). DMA (~119us)
hides fully under compute (~235us). Capacities adapt to the routing at call
time.
"""

from contextlib import ExitStack

import numpy as np
import ml_dtypes

import concourse.bacc as bacc
import concourse.tile as tile
import concourse.mybir as mybir
from concourse.bass_utils import run_bass_kernel_spmd

H = 2048
I = 1024
E = 32
NS = 2
TOP_K = 6
SCALE = 1.0
NCORES = 8
SLOTS = 4          # routed experts per core
TSH = 512          # shared-expert tokens per core (T / 4; 2-way expert split)
KH = H // 128      # 16 k-tiles over H
KI = I // 128      # 8 k-tiles over I
PH = H // 256      # 8 double-row pairs over H
PI = I // 256      # 4 double-row pairs over I
BF16 = mybir.dt.bfloat16
F32 = mybir.dt.float32
FP8 = mybir.dt.float8e4
NP_FP8 = mybir.dt.np(FP8)

# power-of-2 fp8 scales (descales are folded into sigmoid scale / gates).
# fp8e4 here is IEEE e4m3 (max finite 240): z = sigmoid(g) * u carries
# S_WU * S_X = 32x and must stay well under 240 when cast to fp8.
S_X = 8.0          # tokens
S_WG = 8.0         # gate weights
S_WU = 4.0         # up weights
S_WD = 32.0        # down weights
DESCALE_GATE = 1.0 / (S_WG * S_X)                    # on sigmoid input
S_Y = 64.0         # fp8 y-output scale (divided out on host)
DESCALE_Y = S_Y / (S_WU * S_X * S_WD)                # folded into gates

_PROGRAM_CACHE: dict = {}

# experiment knobs (set by bench scripts; kernel() uses the defaults)
ROUTED_MODE = "swil"    # "dr" | "swil"
SHARED_MODE = "hyb"     # "bf16" | "hyb" (fp8 gate/up + bf16 down)
PARTS = "seq"           # phase ordering: routed slots, then shared
WBUFS = 6               # weight tile pool depth
XBUFS = 5               # activation tile pool depth


def _to_bf16(a: np.ndarray) -> np.ndarray:
    """f32 -> bf16 with round-to-nearest-even (fast uint trick)."""
    a = np.ascontiguousarray(a, dtype=np.float32)
    u = a.view(np.uint32)
    r = (u + np.uint32(0x7FFF) + ((u >> np.uint32(16)) & np.uint32(1))) >> np.uint32(16)
    return r.astype(np.uint16).view(ml_dtypes.bfloat16)


def _fp8_pairs(a: np.ndarray, scale: float) -> np.ndarray:
    """[K, N] f32 -> [K/256, 128, 2, N] fp8e4, DoubleRow-interleaved."""
    K, N = a.shape
    q = (np.asarray(a, np.float32) * scale).reshape(K // 256, 2, 128, N)
    return np.ascontiguousarray(q.transpose(0, 2, 1, 3)).astype(NP_FP8)


def _fp8_swil(a: np.ndarray, scale: float) -> np.ndarray:
    """[K, N] f32 -> [K/256, 128, N, 2] fp8e4 for DoubleRowSwInterleave.

    out[q, p, 128*g + jj, b] = a[256q + 128b + p, 128*g + (127 - jj)] * scale
    (A/B pairs interleaved per column; columns reversed within each
    128-col matmul group so LDWEIGHTS reads contiguously.)
    """
    K, N = a.shape
    q = (np.asarray(a, np.float32) * scale).reshape(K // 256, 2, 128, N)
    t = q.transpose(0, 2, 3, 1)                       # [q, p, n, b]
    t = t.reshape(K // 256, 128, N // 128, 128, 2)[:, :, :, ::-1, :]
    return np.ascontiguousarray(t.reshape(K // 256, 128, N, 2)).astype(NP_FP8)


def _route(flat: np.ndarray, Wr: np.ndarray):
    """Host router: softmax over experts, exact top-k gate mask."""
    logits = flat.astype(np.float32) @ Wr.astype(np.float32)
    m = logits.max(axis=-1, keepdims=True)
    p = np.exp(logits - m)
    p /= p.sum(axis=-1, keepdims=True)
    T = p.shape[0]
    idx = np.argpartition(-p, TOP_K - 1, axis=-1)[:, :TOP_K]
    gates = np.zeros((T, E), np.float32)
    rows = np.arange(T)[:, None]
    gates[rows, idx] = p[rows, idx] * SCALE
    return gates


def _assign_experts(tok_idx):
    """Deal experts into (core, slot) balancing per-core token totals.

    Experts with more than 512 tokens (the PSUM-bank N limit) are split into
    pseudo-experts with disjoint token chunks, so slot capacity never exceeds
    512. Slot s holds the pseudo-experts ranked [8s, 8s+8) by token count;
    within a slot the largest goes to the least-loaded core. Returns
    (assign, caps, chunks) where chunks[j] = (expert, token_index_array) and
    assign[core][slot] indexes into chunks (-1 = empty).
    """
    chunks = []
    for e, ix in enumerate(tok_idx):
        for off in range(0, max(len(ix), 1), 512):
            chunks.append((e, ix[off:off + 512]))
    while len(chunks) % NCORES:
        chunks.append((0, np.zeros(0, np.int32)))
    counts = np.array([len(ix) for _, ix in chunks], np.int64)
    n_slots = len(chunks) // NCORES
    order = np.argsort(-counts, kind="stable")
    assign = [[-1] * n_slots for _ in range(NCORES)]
    load = np.zeros(NCORES, np.int64)
    caps = []
    for s in range(n_slots):
        group = list(order[s * NCORES:(s + 1) * NCORES])
        caps.append(int(counts[group].max()) if group else 0)
        for j in group:  # descending count; give to least-loaded core
            c = int(np.argmin(load))
            assign[c][s] = int(j)
            load[c] += counts[j]
    caps = [min(512, max(64, -(-c // 16) * 16)) for c in caps]
    return assign, caps, chunks


def build_program(caps, loop_reps=None, parts="all", routed_mode=None,
                  shared_mode=None):
    """Build the per-core Bass program for the given slot capacities.

    loop_reps: if set, wrap the whole body in a device-side For_i loop —
    used by the test harness to amplify exec time above dispatch overhead.
    routed_mode: "dr" (DoubleRow) | "swil" (DoubleRowSwInterleave weights)
    shared_mode: "bf16" | "hyb" (fp8 gate/up + bf16 down)
    """
    routed_mode = routed_mode or ROUTED_MODE
    shared_mode = shared_mode or SHARED_MODE
    caps = tuple(int(c) for c in caps)
    key = (caps, loop_reps, parts, routed_mode, shared_mode, WBUFS, XBUFS)
    if key in _PROGRAM_CACHE:
        return _PROGRAM_CACHE[key]

    nc = bacc.Bacc("TRN2", target_bir_lowering=False, debug=False)

    swil = routed_mode == "swil"

    def w_shape(KP, N):
        return [KP, 128, N, 2] if swil else [KP, 128, 2, N]

    def w_rearr():
        return "p q i r -> q p i r" if swil else "p q r i -> q p r i"

    def w_slice(t, p, base, width):
        return (t[:, p, base:base + width, :] if swil
                else t[:, p, :, base:base + width])

    def wpool_shape(KP, N):
        return [128, KP, N, 2] if swil else [128, KP, 2, N]

    xg_d, wg_d, wu_d, wd_d, g_d, y_d = [], [], [], [], [], []
    for s in range(len(caps)):
        C = caps[s]
        xg_d.append(nc.dram_tensor(f"xg{s}", [PH, 128, 2, C], FP8, kind="ExternalInput"))
        wg_d.append(nc.dram_tensor(f"wg{s}", w_shape(PH, I), FP8, kind="ExternalInput"))
        wu_d.append(nc.dram_tensor(f"wu{s}", w_shape(PH, I), FP8, kind="ExternalInput"))
        wd_d.append(nc.dram_tensor(f"wd{s}", w_shape(PI, H), FP8, kind="ExternalInput"))
        g_d.append(nc.dram_tensor(f"g{s}", [1, C], BF16, kind="ExternalInput"))
        y_d.append(nc.dram_tensor(f"y{s}", [KH, 128, C], FP8, kind="ExternalOutput"))
    if shared_mode == "hyb":
        xs_d = nc.dram_tensor("xs", [PH, 128, 2, TSH], FP8, kind="ExternalInput")
        wgs_d = nc.dram_tensor("wgs", w_shape(PH, I), FP8, kind="ExternalInput")
        wus_d = nc.dram_tensor("wus", w_shape(PH, I), FP8, kind="ExternalInput")
    else:
        xs_d = nc.dram_tensor("xs", [KH, 128, TSH], BF16, kind="ExternalInput")
        wgs_d = nc.dram_tensor("wgs", [KH, 128, I], BF16, kind="ExternalInput")
        wus_d = nc.dram_tensor("wus", [KH, 128, I], BF16, kind="ExternalInput")
    wds_d = nc.dram_tensor("wds", [KI, 128, H], BF16, kind="ExternalInput")
    ys_d = nc.dram_tensor("ys", [KH, 128, TSH], BF16, kind="ExternalOutput")

    DR = (mybir.MatmulPerfMode.DoubleRowSwInterleave if swil
          else mybir.MatmulPerfMode.DoubleRow)

    with tile.TileContext(nc) as tc:
        with (
            tc.tile_pool(name="w", bufs=WBUFS) as wpool,
            tc.tile_pool(name="xg", bufs=XBUFS) as xpool,
            tc.tile_pool(name="gb", bufs=2) as gbpool,
            tc.tile_pool(name="sg", bufs=2) as sgpool,
            tc.tile_pool(name="z", bufs=2) as zpool,
            tc.tile_pool(name="o", bufs=2) as opool,
            tc.tile_pool(name="os", bufs=2) as ospool,
            tc.tile_pool(name="pg", bufs=2, space="PSUM") as pgpool,
            tc.tile_pool(name="pu", bufs=2, space="PSUM") as pupool,
            tc.tile_pool(name="py", bufs=4, space="PSUM") as pypool,
            ExitStack() as stack,
        ):
            if loop_reps is not None:
                stack.enter_context(tc.For_i(0, loop_reps, 1))

            st = {}

            def routed_expert(s):
                C = caps[s]
                xg_t = xpool.tile([128, PH, 2, C], FP8, tag="xg", name=f"xg_t{s}")
                for hf in range(2):
                    nc.sync.dma_start(
                        xg_t[:, 4 * hf:4 * hf + 4],
                        xg_d[s][4 * hf:4 * hf + 4].rearrange("p q r c -> q p r c"))
                gb = gbpool.tile([128, C], BF16, tag="gb", name=f"gb{s}")
                nc.sync.dma_start(gb[:], g_d[s][:].partition_broadcast(128))

                wg_t = wpool.tile(wpool_shape(PH, I), FP8, tag="w", name=f"wg_t{s}")
                for hf in range(2):
                    nc.sync.dma_start(
                        wg_t[:, 4 * hf:4 * hf + 4],
                        wg_d[s][4 * hf:4 * hf + 4].rearrange(w_rearr()))
                sg = sgpool.tile([128, KI, C], BF16, tag="sg", name=f"sg{s}")
                for m in range(KI):
                    pg = pgpool.tile([128, C], F32, tag="pg", name=f"pg{s}_{m}")
                    for p in range(PH):
                        nc.tensor.matmul(pg[:], w_slice(wg_t, p, m * 128, 128),
                                         xg_t[:, p], start=(p == 0), stop=(p == PH - 1),
                                         perf_mode=DR)
                    nc.scalar.activation(sg[:, m, :], pg[:],
                                         mybir.ActivationFunctionType.Sigmoid,
                                         scale=DESCALE_GATE)

                wu_t = wpool.tile(wpool_shape(PH, I), FP8, tag="w", name=f"wu_t{s}")
                for hf in range(2):
                    nc.sync.dma_start(
                        wu_t[:, 4 * hf:4 * hf + 4],
                        wu_d[s][4 * hf:4 * hf + 4].rearrange(w_rearr()))
                z = zpool.tile([128, KI, C], FP8, tag="z", name=f"z{s}")
                for m in range(KI):
                    pu = pupool.tile([128, C], F32, tag="pu", name=f"pu{s}_{m}")
                    for p in range(PH):
                        nc.tensor.matmul(pu[:], w_slice(wu_t, p, m * 128, 128),
                                         xg_t[:, p], start=(p == 0), stop=(p == PH - 1),
                                         perf_mode=DR)
                    nc.vector.tensor_mul(z[:, m, :], sg[:, m, :], pu[:])

                wd_t = wpool.tile(wpool_shape(PI, H), FP8, tag="w", name=f"wd_t{s}")
                for hf in range(2):
                    nc.sync.dma_start(
                        wd_t[:, 2 * hf:2 * hf + 2],
                        wd_d[s][2 * hf:2 * hf + 2].rearrange(w_rearr()))
                ot = opool.tile([128, KH, C], FP8, tag="o", name=f"ot{s}")
                for h in range(KH):
                    py = pypool.tile([128, C], F32, tag="py", name=f"py{s}_{h}")
                    for p in range(PI):
                        nc.tensor.matmul(py[:], w_slice(wd_t, p, h * 128, 128),
                                         z[:, 2 * p:2 * p + 2, :], start=(p == 0),
                                         stop=(p == PI - 1), perf_mode=DR)
                    nc.vector.tensor_mul(ot[:, h, :], py[:], gb[:])
                for hf in range(2):
                    nc.sync.dma_start(
                        y_d[s][8 * hf:8 * hf + 8].rearrange("h q c -> q h c"),
                        ot[:, 8 * hf:8 * hf + 8])

            # shared-expert sub-phases (bf16, one expert per core half);
            # tiles split into k-halves so each load is an independent DMA
            # in a uniform 16KB/partition weight-pool slot
            def shared_p1():
                if shared_mode == "hyb":
                    xs_t = xpool.tile([128, PH, 2, TSH], FP8, tag="xg",
                                      name="xs_t")
                    for hf in range(2):
                        nc.sync.dma_start(
                            xs_t[:, 4 * hf:4 * hf + 4],
                            xs_d[4 * hf:4 * hf + 4].rearrange("p q r t -> q p r t"))
                    wg_t = wpool.tile(wpool_shape(PH, I), FP8, tag="w",
                                      name="wgs_t")
                    for hf in range(2):
                        nc.sync.dma_start(
                            wg_t[:, 4 * hf:4 * hf + 4],
                            wgs_d[4 * hf:4 * hf + 4].rearrange(w_rearr()))
                    sgs = sgpool.tile([128, KI, TSH], BF16, tag="sg")
                    for m in range(KI):
                        pg = pgpool.tile([128, TSH], F32, tag="pg", name=f"pgs_{m}")
                        for p in range(PH):
                            nc.tensor.matmul(pg[:], w_slice(wg_t, p, m * 128, 128),
                                             xs_t[:, p], start=(p == 0),
                                             stop=(p == PH - 1), perf_mode=DR)
                        nc.scalar.activation(sgs[:, m, :], pg[:],
                                             mybir.ActivationFunctionType.Sigmoid,
                                             scale=DESCALE_GATE)
                    st["xs_t"] = xs_t
                    st["sgs"] = sgs
                    return
                xs_t = [xpool.tile([128, KH // 2, TSH], BF16, tag="xg",
                                   name=f"xs_t{hf}") for hf in range(2)]
                for hf in range(2):
                    nc.sync.dma_start(
                        xs_t[hf][:],
                        xs_d[8 * hf:8 * hf + 8].rearrange("k q t -> q k t"))
                wg_t = [wpool.tile([128, KH // 2, I], BF16, tag="w",
                                   name=f"wgs_t{hf}") for hf in range(2)]
                for hf in range(2):
                    nc.sync.dma_start(
                        wg_t[hf][:],
                        wgs_d[8 * hf:8 * hf + 8].rearrange("k q i -> q k i"))
                sgs = sgpool.tile([128, KI, TSH], BF16, tag="sg")
                for m in range(KI):
                    pg = pgpool.tile([128, TSH], F32, tag="pg", name=f"pgs_{m}")
                    for k in range(KH):
                        nc.tensor.matmul(pg[:],
                                         wg_t[k // 8][:, k % 8, m * 128:(m + 1) * 128],
                                         xs_t[k // 8][:, k % 8, :],
                                         start=(k == 0), stop=(k == KH - 1))
                    nc.scalar.activation(sgs[:, m, :], pg[:],
                                         mybir.ActivationFunctionType.Sigmoid)
                st["xs_t"] = xs_t
                st["sgs"] = sgs

            def shared_p2():
                xs_t, sgs = st["xs_t"], st["sgs"]
                if shared_mode == "hyb":
                    wu_t = wpool.tile(wpool_shape(PH, I), FP8, tag="w",
                                      name="wus_t")
                    for hf in range(2):
                        nc.sync.dma_start(
                            wu_t[:, 4 * hf:4 * hf + 4],
                            wus_d[4 * hf:4 * hf + 4].rearrange(w_rearr()))
                    zb = zpool.tile([128, KI, TSH], BF16, tag="z")
                    for m in range(KI):
                        pu = pupool.tile([128, TSH], F32, tag="pu", name=f"pus_{m}")
                        for p in range(PH):
                            nc.tensor.matmul(pu[:], w_slice(wu_t, p, m * 128, 128),
                                             xs_t[:, p], start=(p == 0),
                                             stop=(p == PH - 1), perf_mode=DR)
                        nc.vector.tensor_mul(zb[:, m, :], sgs[:, m, :], pu[:])
                    st["zb"] = zb
                    return
                wu_t = [wpool.tile([128, KH // 2, I], BF16, tag="w",
                                   name=f"wus_t{hf}") for hf in range(2)]
                for hf in range(2):
                    nc.sync.dma_start(
                        wu_t[hf][:],
                        wus_d[8 * hf:8 * hf + 8].rearrange("k q i -> q k i"))
                zb = zpool.tile([128, KI, TSH], BF16, tag="z")
                for m in range(KI):
                    pu = pupool.tile([128, TSH], F32, tag="pu", name=f"pus_{m}")
                    for k in range(KH):
                        nc.tensor.matmul(pu[:],
                                         wu_t[k // 8][:, k % 8, m * 128:(m + 1) * 128],
                                         xs_t[k // 8][:, k % 8, :],
                                         start=(k == 0), stop=(k == KH - 1))
                    nc.vector.tensor_mul(zb[:, m, :], sgs[:, m, :], pu[:])
                st["zb"] = zb

            def shared_p3(half):
                zb = st["zb"]
                if half == 0:
                    wd_t = [wpool.tile([128, KI // 2, H], BF16, tag="w",
                                       name=f"wds_t{hf}") for hf in range(2)]
                    for hf in range(2):
                        nc.sync.dma_start(
                            wd_t[hf][:],
                            wds_d[4 * hf:4 * hf + 4].rearrange("j q h -> q j h"))
                    st["wd_t"] = wd_t
                wd_t = st["wd_t"]
                for hg in range(2 * half, 2 * half + 2):
                    os_t = ospool.tile([128, 4, TSH], BF16, tag="os",
                                       name=f"os_t{hg}")
                    for hh in range(4):
                        h = hg * 4 + hh
                        py = pypool.tile([128, TSH], F32, tag="py", name=f"pys_{h}")
                        for j in range(KI):
                            nc.tensor.matmul(py[:],
                                             wd_t[j // 4][:, j % 4, h * 128:(h + 1) * 128],
                                             zb[:, j, :], start=(j == 0),
                                             stop=(j == KI - 1))
                        nc.vector.tensor_copy(os_t[:, hh, :], py[:])
                    nc.sync.dma_start(
                        ys_d[hg * 4:(hg + 1) * 4].rearrange("h q t -> q h t"),
                        os_t[:])

            # interleave shared sub-phases between routed experts: shared is
            # PE-bound while routed is DMA-bound, so alternating them lets
            # each phase's idle resource cover the other's busy one
            if parts == "routed":
                steps = [lambda s=s: routed_expert(s) for s in range(len(caps))]
            elif parts == "shared":
                steps = [shared_p1, shared_p2,
                         lambda: shared_p3(0), lambda: shared_p3(1)]
            elif parts == "seq":
                steps = [lambda s=s: routed_expert(s) for s in range(len(caps))]
                steps += [shared_p1, shared_p2,
                          lambda: shared_p3(0), lambda: shared_p3(1)]
            elif parts == "seq2":
                steps = [shared_p1, shared_p2,
                         lambda: shared_p3(0), lambda: shared_p3(1)]
                steps += [lambda s=s: routed_expert(s) for s in range(len(caps))]
            else:
                shared_steps = [shared_p1, shared_p2,
                                lambda: shared_p3(0), lambda: shared_p3(1)]
                steps = []
                for s in range(len(caps)):
                    steps.append(lambda s=s: routed_expert(s))
                    if s < len(shared_steps):
                        steps.append(shared_steps[s])
                steps.extend(shared_steps[len(caps):])
            for step in steps:
                step()

    nc.compile()
    _PROGRAM_CACHE[key] = nc
    return nc


def prepare(x, Wr, Wg_s, Wu_s, Wd_s, Wg, Wu, Wd):
    """Host-side routing, sharding and fp8/bf16 packing. Returns (nc, in_maps, meta)."""
    flat = np.ascontiguousarray(x, np.float32).reshape(-1, H)
    T = flat.shape[0]
    assert T == 4 * TSH

    gates = _route(flat, Wr)
    tok_idx = [np.nonzero(gates[:, e])[0].astype(np.int32) for e in range(E)]
    assign, caps, chunks = _assign_experts(tok_idx)

    nc = build_program(caps, parts=PARTS)

    pack_w = _fp8_swil if ROUTED_MODE == "swil" else _fp8_pairs
    xT = np.ascontiguousarray(flat.T)          # [H, T] f32
    if SHARED_MODE == "hyb":
        wgs_b = [pack_w(np.asarray(Wg_s[e]), S_WG) for e in range(NS)]
        wus_b = [pack_w(np.asarray(Wu_s[e]), S_WU) for e in range(NS)]
        wds_b = [_to_bf16(np.asarray(Wd_s[e]) * (1.0 / (S_X * S_WU)))
                 .reshape(KI, 128, H) for e in range(NS)]
        xs_b = [_fp8_pairs(xT[:, p * TSH:(p + 1) * TSH], S_X)
                for p in range(4)]
    else:
        wgs_b = [_to_bf16(Wg_s[e]).reshape(KH, 128, I) for e in range(NS)]
        wus_b = [_to_bf16(Wu_s[e]).reshape(KH, 128, I) for e in range(NS)]
        wds_b = [_to_bf16(Wd_s[e]).reshape(KI, 128, H) for e in range(NS)]
        xs_b = [_to_bf16(xT[:, p * TSH:(p + 1) * TSH]).reshape(KH, 128, TSH)
                for p in range(4)]

    in_maps = []
    for c in range(NCORES):
        half, part = divmod(c, 4)
        im = {"wgs": wgs_b[half], "wus": wus_b[half], "wds": wds_b[half],
              "xs": xs_b[part]}
        for s in range(len(caps)):
            e, ix = chunks[assign[c][s]]
            C = caps[s]
            xg = np.zeros((H, C), np.float32)
            xg[:, :len(ix)] = xT[:, ix]
            im[f"xg{s}"] = _fp8_pairs(xg, S_X)
            g = np.zeros((1, C), np.float32)
            g[0, :len(ix)] = gates[ix, e] * DESCALE_Y
            im[f"g{s}"] = _to_bf16(g)
            im[f"wg{s}"] = pack_w(np.asarray(Wg[e]), S_WG)
            im[f"wu{s}"] = pack_w(np.asarray(Wu[e]), S_WU)
            im[f"wd{s}"] = pack_w(np.asarray(Wd[e]), S_WD)
        in_maps.append(im)

    meta = {"assign": assign, "caps": caps, "chunks": chunks,
            "flat": flat, "shape": x.shape}
    return nc, in_maps, meta


def postprocess(results, meta):
    """Scatter-add per-expert outputs + shared shards + residual."""
    flat = meta["flat"]
    out = flat.copy()
    for c in range(NCORES):
        part = c % 4
        sh = results[c]["ys"].reshape(H, TSH).astype(np.float32)
        out[part * TSH:(part + 1) * TSH] += sh.T
        for s in range(len(meta["caps"])):
            _, ix = meta["chunks"][meta["assign"][c][s]]
            if len(ix) == 0:
                continue
            Y = results[c][f"y{s}"].reshape(H, meta["caps"][s])
            out[ix] += Y[:, :len(ix)].T.astype(np.float32) * (1.0 / S_Y)
    return out.reshape(meta["shape"]).astype(np.float32, copy=False)


def kernel(x, Wr, Wg_s, Wu_s, Wd_s, Wg, Wu, Wd):
    nc, in_maps, meta = prepare(x, Wr, Wg_s, Wu_s, Wd_s, Wg, Wu, Wd)
    last_err = None
    for _ in range(3):  # the tunneled device occasionally drops a run
        try:
            res = run_bass_kernel_spmd(nc, in_maps, list(range(NCORES)))
            return postprocess(res.results, meta)
        except Exception as err:  # noqa: BLE001
            last_err = err
    raise last_err



# revision 19
# speedup vs baseline: 1.1696x; 1.0092x over previous
"""Trainium2 Bass kernel for BaselineMoE (top-6-of-32 routed experts + 2 shared).

Strategy (8 NeuronCores, expert-parallel per the sharding hint):
  - Host computes the (cheap) router softmax/top-k from the actual inputs,
    gathers each expert's tokens into a padded, transposed buffer, and deals
    the 32 routed experts across 8 cores x 4 slots, balancing per-core load
    (sum of slot capacities is provably minimal for 4 slots x 8 cores).
  - Each core runs a dense SwiGLU MLP (gate/up/down, sigmoid(gate)*up) for its
    4 routed experts on the pre-gathered tokens, with the per-token top-k gate
    weights applied on-device during PSUM evacuation.
  - Routed experts run in fp8e4 with DoubleRowSwInterleave matmuls (2
    contraction rows per PE cell, software-interleaved weight layout) using
    power-of-2 scales folded into the sigmoid input scale and the gate
    coefficients; PSUM accumulation stays f32.
  - The 2 shared experts are split across core halves (cores 0-3 run shared
    expert 0, cores 4-7 expert 1, each on a 512-token shard). Their gate/up
    matmuls run in fp8 DoubleRow (the sigmoid and the z = sig(g)*u product
    damp the quantization error); the down projection stays bf16, which keeps
    total rel-err ~1.6e-2 vs the 2e-2 budget (measured on HW). The 1/32
    activation descale is folded into the bf16 down weights host-side.
  - Phases run sequentially (all routed slots, then shared): measured ~7us
    faster than interleaving because shared tiles otherwise occupy weight-pool
    buffers across routed phases and starve the DMA prefetch depth.
  - Each tensor is loaded/stored with a few large rearranged-AP DMAs to keep
    the DGE issue path off the critical path. Expert outputs come back fp8
    (scaled by S_Y); the host scatter-adds them into the residual in f32.

Perf model (measured on these cores): PE streams ~0.54ns per output column
for both bf16 and fp8-DR, so time ~= total matmul output columns x 0.54ns;
fp8-DR halves the column count per flop (256-deep contraction). DMA (~119us)
hides fully under compute (~235us). Capacities adapt to the routing at call
time.
"""

from contextlib import ExitStack

import numpy as np
import ml_dtypes

import concourse.bacc as bacc
import concourse.tile as tile
import concourse.mybir as mybir
from concourse.bass_utils import run_bass_kernel_spmd

H = 2048
I = 1024
E = 32
NS = 2
TOP_K = 6
SCALE = 1.0
NCORES = 8
SLOTS = 4          # routed experts per core
TSH = 512          # shared-expert tokens per core (T / 4; 2-way expert split)
KH = H // 128      # 16 k-tiles over H
KI = I // 128      # 8 k-tiles over I
PH = H // 256      # 8 double-row pairs over H
PI = I // 256      # 4 double-row pairs over I
BF16 = mybir.dt.bfloat16
F32 = mybir.dt.float32
FP8 = mybir.dt.float8e4
NP_FP8 = mybir.dt.np(FP8)

# power-of-2 fp8 scales (descales are folded into sigmoid scale / gates).
# fp8e4 here is IEEE e4m3 (max finite 240): z = sigmoid(g) * u carries
# S_WU * S_X = 32x and must stay well under 240 when cast to fp8.
S_X = 8.0          # tokens
S_WG = 8.0         # gate weights
S_WU = 4.0         # up weights
S_WD = 32.0        # down weights
DESCALE_GATE = 1.0 / (S_WG * S_X)                    # on sigmoid input
S_Y = 64.0         # fp8 y-output scale (divided out on host)
DESCALE_Y = S_Y / (S_WU * S_X * S_WD)                # folded into gates

_PROGRAM_CACHE: dict = {}

# experiment knobs (set by bench scripts; kernel() uses the defaults)
ROUTED_MODE = "swil"    # "dr" | "swil"
SHARED_MODE = "hyb"     # "bf16" | "hyb" (fp8 gate/up + bf16 down)
PARTS = "seq"           # phase ordering: routed slots, then shared
WBUFS = 6               # weight tile pool depth
XBUFS = 5               # activation tile pool depth
DSPLIT = 2              # DMA chunks per routed input tensor


def _to_bf16(a: np.ndarray) -> np.ndarray:
    """f32 -> bf16 with round-to-nearest-even (fast uint trick)."""
    a = np.ascontiguousarray(a, dtype=np.float32)
    u = a.view(np.uint32)
    r = (u + np.uint32(0x7FFF) + ((u >> np.uint32(16)) & np.uint32(1))) >> np.uint32(16)
    return r.astype(np.uint16).view(ml_dtypes.bfloat16)


def _fp8_pairs(a: np.ndarray, scale: float) -> np.ndarray:
    """[K, N] f32 -> [K/256, 128, 2, N] fp8e4, DoubleRow-interleaved."""
    K, N = a.shape
    q = (np.asarray(a, np.float32) * scale).reshape(K // 256, 2, 128, N)
    return np.ascontiguousarray(q.transpose(0, 2, 1, 3)).astype(NP_FP8)


def _fp8_swil(a: np.ndarray, scale: float) -> np.ndarray:
    """[K, N] f32 -> [K/256, 128, N, 2] fp8e4 for DoubleRowSwInterleave.

    out[q, p, 128*g + jj, b] = a[256q + 128b + p, 128*g + (127 - jj)] * scale
    (A/B pairs interleaved per column; columns reversed within each
    128-col matmul group so LDWEIGHTS reads contiguously.)
    """
    K, N = a.shape
    q = (np.asarray(a, np.float32) * scale).reshape(K // 256, 2, 128, N)
    t = q.transpose(0, 2, 3, 1)                       # [q, p, n, b]
    t = t.reshape(K // 256, 128, N // 128, 128, 2)[:, :, :, ::-1, :]
    return np.ascontiguousarray(t.reshape(K // 256, 128, N, 2)).astype(NP_FP8)


def _route(flat: np.ndarray, Wr: np.ndarray):
    """Host router: softmax over experts, exact top-k gate mask."""
    logits = flat.astype(np.float32) @ Wr.astype(np.float32)
    m = logits.max(axis=-1, keepdims=True)
    p = np.exp(logits - m)
    p /= p.sum(axis=-1, keepdims=True)
    T = p.shape[0]
    idx = np.argpartition(-p, TOP_K - 1, axis=-1)[:, :TOP_K]
    gates = np.zeros((T, E), np.float32)
    rows = np.arange(T)[:, None]
    gates[rows, idx] = p[rows, idx] * SCALE
    return gates


def _assign_experts(tok_idx):
    """Deal experts into (core, slot) balancing per-core token totals.

    Experts with more than 512 tokens (the PSUM-bank N limit) are split into
    pseudo-experts with disjoint token chunks, so slot capacity never exceeds
    512. Slot s holds the pseudo-experts ranked [8s, 8s+8) by token count;
    within a slot the largest goes to the least-loaded core. Returns
    (assign, caps, chunks) where chunks[j] = (expert, token_index_array) and
    assign[core][slot] indexes into chunks (-1 = empty).
    """
    chunks = []
    for e, ix in enumerate(tok_idx):
        for off in range(0, max(len(ix), 1), 512):
            chunks.append((e, ix[off:off + 512]))
    while len(chunks) % NCORES:
        chunks.append((0, np.zeros(0, np.int32)))
    counts = np.array([len(ix) for _, ix in chunks], np.int64)
    n_slots = len(chunks) // NCORES
    order = np.argsort(-counts, kind="stable")
    assign = [[-1] * n_slots for _ in range(NCORES)]
    load = np.zeros(NCORES, np.int64)
    caps = []
    for s in range(n_slots):
        group = list(order[s * NCORES:(s + 1) * NCORES])
        caps.append(int(counts[group].max()) if group else 0)
        for j in group:  # descending count; give to least-loaded core
            c = int(np.argmin(load))
            assign[c][s] = int(j)
            load[c] += counts[j]
    caps = [min(512, max(64, -(-c // 16) * 16)) for c in caps]
    return assign, caps, chunks


def build_program(caps, loop_reps=None, parts="all", routed_mode=None,
                  shared_mode=None):
    """Build the per-core Bass program for the given slot capacities.

    loop_reps: if set, wrap the whole body in a device-side For_i loop —
    used by the test harness to amplify exec time above dispatch overhead.
    routed_mode: "dr" (DoubleRow) | "swil" (DoubleRowSwInterleave weights)
    shared_mode: "bf16" | "hyb" (fp8 gate/up + bf16 down)
    """
    routed_mode = routed_mode or ROUTED_MODE
    shared_mode = shared_mode or SHARED_MODE
    caps = tuple(int(c) for c in caps)
    key = (caps, loop_reps, parts, routed_mode, shared_mode, WBUFS, XBUFS,
           DSPLIT)
    if key in _PROGRAM_CACHE:
        return _PROGRAM_CACHE[key]

    nc = bacc.Bacc("TRN2", target_bir_lowering=False, debug=False)

    swil = routed_mode == "swil"

    def w_shape(KP, N):
        return [KP, 128, N, 2] if swil else [KP, 128, 2, N]

    def w_rearr():
        return "p q i r -> q p i r" if swil else "p q r i -> q p r i"

    def w_slice(t, p, base, width):
        return (t[:, p, base:base + width, :] if swil
                else t[:, p, :, base:base + width])

    def wpool_shape(KP, N):
        return [128, KP, N, 2] if swil else [128, KP, 2, N]

    xg_d, wg_d, wu_d, wd_d, g_d, y_d = [], [], [], [], [], []
    for s in range(len(caps)):
        C = caps[s]
        xg_d.append(nc.dram_tensor(f"xg{s}", [PH, 128, 2, C], FP8, kind="ExternalInput"))
        wg_d.append(nc.dram_tensor(f"wg{s}", w_shape(PH, I), FP8, kind="ExternalInput"))
        wu_d.append(nc.dram_tensor(f"wu{s}", w_shape(PH, I), FP8, kind="ExternalInput"))
        wd_d.append(nc.dram_tensor(f"wd{s}", w_shape(PI, H), FP8, kind="ExternalInput"))
        g_d.append(nc.dram_tensor(f"g{s}", [1, C], BF16, kind="ExternalInput"))
        y_d.append(nc.dram_tensor(f"y{s}", [KH, 128, C], FP8, kind="ExternalOutput"))
    if shared_mode == "hyb":
        xs_d = nc.dram_tensor("xs", [PH, 128, 2, TSH], FP8, kind="ExternalInput")
        wgs_d = nc.dram_tensor("wgs", w_shape(PH, I), FP8, kind="ExternalInput")
        wus_d = nc.dram_tensor("wus", w_shape(PH, I), FP8, kind="ExternalInput")
    else:
        xs_d = nc.dram_tensor("xs", [KH, 128, TSH], BF16, kind="ExternalInput")
        wgs_d = nc.dram_tensor("wgs", [KH, 128, I], BF16, kind="ExternalInput")
        wus_d = nc.dram_tensor("wus", [KH, 128, I], BF16, kind="ExternalInput")
    wds_d = nc.dram_tensor("wds", [KI, 128, H], BF16, kind="ExternalInput")
    ys_d = nc.dram_tensor("ys", [KH, 128, TSH], BF16, kind="ExternalOutput")

    DR = (mybir.MatmulPerfMode.DoubleRowSwInterleave if swil
          else mybir.MatmulPerfMode.DoubleRow)

    with tile.TileContext(nc) as tc:
        with (
            tc.tile_pool(name="w", bufs=WBUFS) as wpool,
            tc.tile_pool(name="xg", bufs=XBUFS) as xpool,
            tc.tile_pool(name="gb", bufs=2) as gbpool,
            tc.tile_pool(name="sg", bufs=2) as sgpool,
            tc.tile_pool(name="z", bufs=2) as zpool,
            tc.tile_pool(name="o", bufs=2) as opool,
            tc.tile_pool(name="os", bufs=2) as ospool,
            tc.tile_pool(name="pg", bufs=2, space="PSUM") as pgpool,
            tc.tile_pool(name="pu", bufs=2, space="PSUM") as pupool,
            tc.tile_pool(name="py", bufs=4, space="PSUM") as pypool,
            ExitStack() as stack,
        ):
            if loop_reps is not None:
                stack.enter_context(tc.For_i(0, loop_reps, 1))

            st = {}

            def routed_expert(s):
                C = caps[s]
                nsp = PH // DSPLIT
                xg_t = xpool.tile([128, PH, 2, C], FP8, tag="xg", name=f"xg_t{s}")
                for hf in range(DSPLIT):
                    nc.sync.dma_start(
                        xg_t[:, nsp * hf:nsp * (hf + 1)],
                        xg_d[s][nsp * hf:nsp * (hf + 1)].rearrange("p q r c -> q p r c"))
                gb = gbpool.tile([128, C], BF16, tag="gb", name=f"gb{s}")
                nc.sync.dma_start(gb[:], g_d[s][:].partition_broadcast(128))

                wg_t = wpool.tile(wpool_shape(PH, I), FP8, tag="w", name=f"wg_t{s}")
                for hf in range(DSPLIT):
                    nc.sync.dma_start(
                        wg_t[:, nsp * hf:nsp * (hf + 1)],
                        wg_d[s][nsp * hf:nsp * (hf + 1)].rearrange(w_rearr()))
                sg = sgpool.tile([128, KI, C], BF16, tag="sg", name=f"sg{s}")
                for m in range(KI):
                    pg = pgpool.tile([128, C], F32, tag="pg", name=f"pg{s}_{m}")
                    for p in range(PH):
                        nc.tensor.matmul(pg[:], w_slice(wg_t, p, m * 128, 128),
                                         xg_t[:, p], start=(p == 0), stop=(p == PH - 1),
                                         perf_mode=DR)
                    nc.scalar.activation(sg[:, m, :], pg[:],
                                         mybir.ActivationFunctionType.Sigmoid,
                                         scale=DESCALE_GATE)

                wu_t = wpool.tile(wpool_shape(PH, I), FP8, tag="w", name=f"wu_t{s}")
                for hf in range(DSPLIT):
                    nc.sync.dma_start(
                        wu_t[:, nsp * hf:nsp * (hf + 1)],
                        wu_d[s][nsp * hf:nsp * (hf + 1)].rearrange(w_rearr()))
                z = zpool.tile([128, KI, C], FP8, tag="z", name=f"z{s}")
                for m in range(KI):
                    pu = pupool.tile([128, C], F32, tag="pu", name=f"pu{s}_{m}")
                    for p in range(PH):
                        nc.tensor.matmul(pu[:], w_slice(wu_t, p, m * 128, 128),
                                         xg_t[:, p], start=(p == 0), stop=(p == PH - 1),
                                         perf_mode=DR)
                    nc.vector.tensor_mul(z[:, m, :], sg[:, m, :], pu[:])

                wd_t = wpool.tile(wpool_shape(PI, H), FP8, tag="w", name=f"wd_t{s}")
                for hf in range(2):
                    nc.sync.dma_start(
                        wd_t[:, 2 * hf:2 * hf + 2],
                        wd_d[s][2 * hf:2 * hf + 2].rearrange(w_rearr()))
                ot = opool.tile([128, KH, C], FP8, tag="o", name=f"ot{s}")
                for h in range(KH):
                    py = pypool.tile([128, C], F32, tag="py", name=f"py{s}_{h}")
                    for p in range(PI):
                        nc.tensor.matmul(py[:], w_slice(wd_t, p, h * 128, 128),
                                         z[:, 2 * p:2 * p + 2, :], start=(p == 0),
                                         stop=(p == PI - 1), perf_mode=DR)
                    nc.vector.tensor_mul(ot[:, h, :], py[:], gb[:])
                for hf in range(2):
                    nc.sync.dma_start(
                        y_d[s][8 * hf:8 * hf + 8].rearrange("h q c -> q h c"),
                        ot[:, 8 * hf:8 * hf + 8])

            # shared-expert sub-phases (bf16, one expert per core half);
            # tiles split into k-halves so each load is an independent DMA
            # in a uniform 16KB/partition weight-pool slot
            def shared_p1():
                if shared_mode == "hyb":
                    xs_t = xpool.tile([128, PH, 2, TSH], FP8, tag="xg",
                                      name="xs_t")
                    for hf in range(2):
                        nc.sync.dma_start(
                            xs_t[:, 4 * hf:4 * hf + 4],
                            xs_d[4 * hf:4 * hf + 4].rearrange("p q r t -> q p r t"))
                    wg_t = wpool.tile(wpool_shape(PH, I), FP8, tag="w",
                                      name="wgs_t")
                    for hf in range(2):
                        nc.sync.dma_start(
                            wg_t[:, 4 * hf:4 * hf + 4],
                            wgs_d[4 * hf:4 * hf + 4].rearrange(w_rearr()))
                    sgs = sgpool.tile([128, KI, TSH], BF16, tag="sg")
                    for m in range(KI):
                        pg = pgpool.tile([128, TSH], F32, tag="pg", name=f"pgs_{m}")
                        for p in range(PH):
                            nc.tensor.matmul(pg[:], w_slice(wg_t, p, m * 128, 128),
                                             xs_t[:, p], start=(p == 0),
                                             stop=(p == PH - 1), perf_mode=DR)
                        nc.scalar.activation(sgs[:, m, :], pg[:],
                                             mybir.ActivationFunctionType.Sigmoid,
                                             scale=DESCALE_GATE)
                    st["xs_t"] = xs_t
                    st["sgs"] = sgs
                    return
                xs_t = [xpool.tile([128, KH // 2, TSH], BF16, tag="xg",
                                   name=f"xs_t{hf}") for hf in range(2)]
                for hf in range(2):
                    nc.sync.dma_start(
                        xs_t[hf][:],
                        xs_d[8 * hf:8 * hf + 8].rearrange("k q t -> q k t"))
                wg_t = [wpool.tile([128, KH // 2, I], BF16, tag="w",
                                   name=f"wgs_t{hf}") for hf in range(2)]
                for hf in range(2):
                    nc.sync.dma_start(
                        wg_t[hf][:],
                        wgs_d[8 * hf:8 * hf + 8].rearrange("k q i -> q k i"))
                sgs = sgpool.tile([128, KI, TSH], BF16, tag="sg")
                for m in range(KI):
                    pg = pgpool.tile([128, TSH], F32, tag="pg", name=f"pgs_{m}")
                    for k in range(KH):
                        nc.tensor.matmul(pg[:],
                                         wg_t[k // 8][:, k % 8, m * 128:(m + 1) * 128],
                                         xs_t[k // 8][:, k % 8, :],
                                         start=(k == 0), stop=(k == KH - 1))
                    nc.scalar.activation(sgs[:, m, :], pg[:],
                                         mybir.ActivationFunctionType.Sigmoid)
                st["xs_t"] = xs_t
                st["sgs"] = sgs

            def shared_p2():
                xs_t, sgs = st["xs_t"], st["sgs"]
                if shared_mode == "hyb":
                    wu_t = wpool.tile(wpool_shape(PH, I), FP8, tag="w",
                                      name="wus_t")
                    for hf in range(2):
                        nc.sync.dma_start(
                            wu_t[:, 4 * hf:4 * hf + 4],
                            wus_d[4 * hf:4 * hf + 4].rearrange(w_rearr()))
                    zb = zpool.tile([128, KI, TSH], BF16, tag="z")
                    for m in range(KI):
                        pu = pupool.tile([128, TSH], F32, tag="pu", name=f"pus_{m}")
                        for p in range(PH):
                            nc.tensor.matmul(pu[:], w_slice(wu_t, p, m * 128, 128),
                                             xs_t[:, p], start=(p == 0),
                                             stop=(p == PH - 1), perf_mode=DR)
                        nc.vector.tensor_mul(zb[:, m, :], sgs[:, m, :], pu[:])
                    st["zb"] = zb
                    return
                wu_t = [wpool.tile([128, KH // 2, I], BF16, tag="w",
                                   name=f"wus_t{hf}") for hf in range(2)]
                for hf in range(2):
                    nc.sync.dma_start(
                        wu_t[hf][:],
                        wus_d[8 * hf:8 * hf + 8].rearrange("k q i -> q k i"))
                zb = zpool.tile([128, KI, TSH], BF16, tag="z")
                for m in range(KI):
                    pu = pupool.tile([128, TSH], F32, tag="pu", name=f"pus_{m}")
                    for k in range(KH):
                        nc.tensor.matmul(pu[:],
                                         wu_t[k // 8][:, k % 8, m * 128:(m + 1) * 128],
                                         xs_t[k // 8][:, k % 8, :],
                                         start=(k == 0), stop=(k == KH - 1))
                    nc.vector.tensor_mul(zb[:, m, :], sgs[:, m, :], pu[:])
                st["zb"] = zb

            def shared_p3(half):
                zb = st["zb"]
                if half == 0:
                    wd_t = [wpool.tile([128, KI // 2, H], BF16, tag="w",
                                       name=f"wds_t{hf}") for hf in range(2)]
                    for hf in range(2):
                        nc.sync.dma_start(
                            wd_t[hf][:],
                            wds_d[4 * hf:4 * hf + 4].rearrange("j q h -> q j h"))
                    st["wd_t"] = wd_t
                wd_t = st["wd_t"]
                for hg in range(2 * half, 2 * half + 2):
                    os_t = ospool.tile([128, 4, TSH], BF16, tag="os",
                                       name=f"os_t{hg}")
                    for hh in range(4):
                        h = hg * 4 + hh
                        py = pypool.tile([128, TSH], F32, tag="py", name=f"pys_{h}")
                        for j in range(KI):
                            nc.tensor.matmul(py[:],
                                             wd_t[j // 4][:, j % 4, h * 128:(h + 1) * 128],
                                             zb[:, j, :], start=(j == 0),
                                             stop=(j == KI - 1))
                        nc.vector.tensor_copy(os_t[:, hh, :], py[:])
                    nc.sync.dma_start(
                        ys_d[hg * 4:(hg + 1) * 4].rearrange("h q t -> q h t"),
                        os_t[:])

            # interleave shared sub-phases between routed experts: shared is
            # PE-bound while routed is DMA-bound, so alternating them lets
            # each phase's idle resource cover the other's busy one
            if parts == "routed":
                steps = [lambda s=s: routed_expert(s) for s in range(len(caps))]
            elif parts == "shared":
                steps = [shared_p1, shared_p2,
                         lambda: shared_p3(0), lambda: shared_p3(1)]
            elif parts == "seq":
                steps = [lambda s=s: routed_expert(s) for s in range(len(caps))]
                steps += [shared_p1, shared_p2,
                          lambda: shared_p3(0), lambda: shared_p3(1)]
            elif parts == "seq2":
                steps = [shared_p1, shared_p2,
                         lambda: shared_p3(0), lambda: shared_p3(1)]
                steps += [lambda s=s: routed_expert(s) for s in range(len(caps))]
            else:
                shared_steps = [shared_p1, shared_p2,
                                lambda: shared_p3(0), lambda: shared_p3(1)]
                steps = []
                for s in range(len(caps)):
                    steps.append(lambda s=s: routed_expert(s))
                    if s < len(shared_steps):
                        steps.append(shared_steps[s])
                steps.extend(shared_steps[len(caps):])
            for step in steps:
                step()

    nc.compile()
    _PROGRAM_CACHE[key] = nc
    return nc


def prepare(x, Wr, Wg_s, Wu_s, Wd_s, Wg, Wu, Wd):
    """Host-side routing, sharding and fp8/bf16 packing. Returns (nc, in_maps, meta)."""
    flat = np.ascontiguousarray(x, np.float32).reshape(-1, H)
    T = flat.shape[0]
    assert T == 4 * TSH

    gates = _route(flat, Wr)
    tok_idx = [np.nonzero(gates[:, e])[0].astype(np.int32) for e in range(E)]
    assign, caps, chunks = _assign_experts(tok_idx)

    nc = build_program(caps, parts=PARTS)

    pack_w = _fp8_swil if ROUTED_MODE == "swil" else _fp8_pairs
    xT = np.ascontiguousarray(flat.T)          # [H, T] f32
    if SHARED_MODE == "hyb":
        wgs_b = [pack_w(np.asarray(Wg_s[e]), S_WG) for e in range(NS)]
        wus_b = [pack_w(np.asarray(Wu_s[e]), S_WU) for e in range(NS)]
        wds_b = [_to_bf16(np.asarray(Wd_s[e]) * (1.0 / (S_X * S_WU)))
                 .reshape(KI, 128, H) for e in range(NS)]
        xs_b = [_fp8_pairs(xT[:, p * TSH:(p + 1) * TSH], S_X)
                for p in range(4)]
    else:
        wgs_b = [_to_bf16(Wg_s[e]).reshape(KH, 128, I) for e in range(NS)]
        wus_b = [_to_bf16(Wu_s[e]).reshape(KH, 128, I) for e in range(NS)]
        wds_b = [_to_bf16(Wd_s[e]).reshape(KI, 128, H) for e in range(NS)]
        xs_b = [_to_bf16(xT[:, p * TSH:(p + 1) * TSH]).reshape(KH, 128, TSH)
                for p in range(4)]

    in_maps = []
    for c in range(NCORES):
        half, part = divmod(c, 4)
        im = {"wgs": wgs_b[half], "wus": wus_b[half], "wds": wds_b[half],
              "xs": xs_b[part]}
        for s in range(len(caps)):
            e, ix = chunks[assign[c][s]]
            C = caps[s]
            xg = np.zeros((H, C), np.float32)
            xg[:, :len(ix)] = xT[:, ix]
            im[f"xg{s}"] = _fp8_pairs(xg, S_X)
            g = np.zeros((1, C), np.float32)
            g[0, :len(ix)] = gates[ix, e] * DESCALE_Y
            im[f"g{s}"] = _to_bf16(g)
            im[f"wg{s}"] = pack_w(np.asarray(Wg[e]), S_WG)
            im[f"wu{s}"] = pack_w(np.asarray(Wu[e]), S_WU)
            im[f"wd{s}"] = pack_w(np.asarray(Wd[e]), S_WD)
        in_maps.append(im)

    meta = {"assign": assign, "caps": caps, "chunks": chunks,
            "flat": flat, "shape": x.shape}
    return nc, in_maps, meta


def postprocess(results, meta):
    """Scatter-add per-expert outputs + shared shards + residual."""
    flat = meta["flat"]
    out = flat.copy()
    for c in range(NCORES):
        part = c % 4
        sh = results[c]["ys"].reshape(H, TSH).astype(np.float32)
        out[part * TSH:(part + 1) * TSH] += sh.T
        for s in range(len(meta["caps"])):
            _, ix = meta["chunks"][meta["assign"][c][s]]
            if len(ix) == 0:
                continue
            Y = results[c][f"y{s}"].reshape(H, meta["caps"][s])
            out[ix] += Y[:, :len(ix)].T.astype(np.float32) * (1.0 / S_Y)
    return out.reshape(meta["shape"]).astype(np.float32, copy=False)


def kernel(x, Wr, Wg_s, Wu_s, Wd_s, Wg, Wu, Wd):
    nc, in_maps, meta = prepare(x, Wr, Wg_s, Wu_s, Wd_s, Wg, Wu, Wd)
    last_err = None
    for _ in range(3):  # the tunneled device occasionally drops a run
        try:
            res = run_bass_kernel_spmd(nc, in_maps, list(range(NCORES)))
            return postprocess(res.results, meta)
        except Exception as err:  # noqa: BLE001
            last_err = err
    raise last_err

